# revision 1
# baseline (speedup 1.0000x reference)
"""Causal self-attention Bass/Tile kernel for 8 TRN2 NeuronCores.

Sharding: core c handles batch b = c//2 and heads h in [8*(c%2), 8*(c%2)+8).
Each core computes a partial projection output (its 512 channels' worth of the
contraction); the host sums the two partials per batch.

Per-core pipeline (per 512-wide t-chunk):
  xT  [c,t] bf16   <- hardware DMA-transpose of host-cast bf16 x
  wT  [c,j] bf16   <- DMA-transpose of host-cast bf16 wqkv (one-time)
  q,k [j,t] f32r   <- bf16 matmul (lhsT=wT, rhs=xT), psum->sbuf cast
  v   [t,j] bf16   <- bf16 matmul (lhsT=xT, rhs=wT_v), with ones column for l
  S.T [tk,tq] psum <- f32r matmul, 2 heads row-tiled; causal mask pre-added
                      into psum via identity@mask matmul on diagonal tiles
  P = exp(S/8)     <- single ACT call per tk-tile (both heads), bf16 out
  y.T|l  psum      <- bf16 AV matmul accumulation; scores emitted one tile
                      ahead of AV so ACT never stalls the PE queue
  y.T/l            <- l broadcast via K=2 select-matmul, DVE divide
  out [t,c] f32    <- f32r proj matmul from y.T tiles, DMA to DRAM
"""

import sys

if "/opt/trn_rl_repo" not in sys.path:
    sys.path.insert(0, "/opt/trn_rl_repo")

import ml_dtypes
import numpy as np

import concourse.bass as bass
import concourse.mybir as mybir
import concourse.tile as tile
from concourse import bacc, bass_utils
from concourse.masks import make_identity

F32 = mybir.dt.float32
F32R = mybir.dt.float32r
BF16 = mybir.dt.bfloat16

B, T, C = 4, 2048, 1024
H = 16
D = 64
JL = 512          # local channels per q/k/v slice (8 heads * 64)
P = 128
NCHUNK = T // 512
NPAIR = 4
NEG = -1e30


def build_nc():
    nc = bacc.Bacc("TRN2", target_bir_lowering=False, debug=False)
    xb = nc.dram_tensor("xb", [T, C], BF16, kind="ExternalInput").ap()
    wqkvb = nc.dram_tensor("wqkvb", [3 * JL, C], BF16, kind="ExternalInput").ap()
    wproj = nc.dram_tensor("wproj", [C, JL], F32, kind="ExternalInput").ap()
    out = nc.dram_tensor("out", [T, C], F32, kind="ExternalOutput").ap()

    CT = C // P       # 8 c-tiles
    Exp = mybir.ActivationFunctionType.Exp
    Copy = mybir.ActivationFunctionType.Copy

    with tile.TileContext(nc) as tc:
        with (
            tc.tile_pool(name="singles", bufs=1) as singles,
            tc.tile_pool(name="stage", bufs=2) as stage_pool,
            tc.tile_pool(name="xT", bufs=2) as xT_pool,
            tc.tile_pool(name="qsb", bufs=2) as qsb_pool,
            tc.tile_pool(name="pt", bufs=4) as pt_pool,
            tc.tile_pool(name="yT", bufs=2) as yT_pool,
            tc.tile_pool(name="ob", bufs=3) as ob_pool,
            tc.tile_pool(name="small", bufs=3) as small_pool,
            tc.tile_pool(name="ps_sc", bufs=2, space="PSUM") as ps_sc,
            tc.tile_pool(name="ps_av", bufs=2, space="PSUM") as ps_av,
            tc.tile_pool(name="ps_mm", bufs=2, space="PSUM") as ps_mm,
        ):
            identity = singles.tile([P, P], F32)
            make_identity(nc, identity)
            identity_r = singles.tile([P, P], F32R)
            nc.vector.tensor_copy(identity_r, identity)

            # head-pair selector for the l broadcast: out rows 0:64 <- l0,
            # rows 64:128 <- l1  (out = sel2.T @ [l0;l1])
            selaf = singles.tile([1, P], F32)
            nc.vector.memset(selaf, 0.0)
            nc.vector.memset(selaf[0:1, 0:D], 1.0)
            selbf = singles.tile([1, P], F32)
            nc.vector.memset(selbf, 0.0)
            nc.vector.memset(selbf[0:1, D:2 * D], 1.0)
            sel_a = singles.tile([1, P], F32R)
            nc.vector.tensor_copy(sel_a, selaf)
            sel_b = singles.tile([1, P], F32R)
            nc.vector.tensor_copy(sel_b, selbf)

            # additive causal masks, one per diagonal offset o=128*k:
            # mask[k][p, f] = 0 if f >= p + 128k else -1e30   (f in 0..511)
            maskf = singles.tile([P, 4, 512], F32)
            for k in range(4):
                nc.gpsimd.memset(maskf[:, k, :], 0.0)
                nc.gpsimd.affine_select(
                    out=maskf[:, k, :],
                    in_=maskf[:, k, :],
                    pattern=[[1, 512]],
                    compare_op=mybir.AluOpType.is_ge,
                    fill=NEG,
                    base=-(128 * k),
                    channel_multiplier=-1,
                )
            maskr = singles.tile([P, 4, 512], F32R)
            nc.vector.tensor_copy(maskr, maskf)

            # persistent tensors
            wT = singles.tile([P, 3, CT, JL], BF16)   # [c, qkv, cc, j] 24KB/part
            wprojT = singles.tile([P, 4, C], F32R)    # [j, g, c_out]  16KB/part
            k_sb = singles.tile([P, 4, T], F32R)      # [d2, hp, tk]   32KB/part
            v_sb = singles.tile([P, T // P, 8, D + 1], BF16)  # [t, tkt, h, d|1]
            nc.vector.memset(v_sb[:, :, :, D], 1.0)

            # ---- stage 0: weight loads (DMA transpose for bf16 wqkv) ----
            for g3 in range(3):
                for cc in range(CT):
                    nc.sync.dma_start_transpose(
                        wT[:, g3, cc, :],
                        wqkvb[g3 * JL:(g3 + 1) * JL, cc * P:(cc + 1) * P],
                    )
            # wproj via PE transpose (fp32 path, one-time 32 tiles)
            for ct in range(8):
                wp = stage_pool.tile([P, JL], F32, tag="wp")
                nc.sync.dma_start(out=wp, in_=wproj[ct * P:(ct + 1) * P, :])
                for g in range(4):
                    ptile = ps_mm.tile([P, P], F32, tag="mm")
                    nc.tensor.transpose(ptile, wp[:, g * P:(g + 1) * P], identity)
                    nc.vector.tensor_copy(
                        out=wprojT[:, g, ct * P:(ct + 1) * P], in_=ptile
                    )

            pending_div = None

            def emit_div(pend):
                pav0, pav1, yT_t, hp_ = pend
                # l rows (psum row 64) -> sbuf f32r via ACT: its queue sits
                # right behind the pair's last exp so the wait is short
                l2a = small_pool.tile([1, 512], F32R, tag="l2a")
                l2b = small_pool.tile([1, 512], F32R, tag="l2b")
                nc.scalar.activation(l2a, pav0[D:D + 1, :], Copy)
                nc.scalar.activation(l2b, pav1[D:D + 1, :], Copy)
                pb = ps_mm.tile([P, 512], F32, tag="mm")
                nc.tensor.matmul(pb, lhsT=sel_a, rhs=l2a,
                                 start=True, stop=False)
                nc.tensor.matmul(pb, lhsT=sel_b, rhs=l2b,
                                 start=False, stop=True)
                pbs = small_pool.tile([P, 512], F32, tag="pbs")
                nc.vector.reciprocal_approx_fast(out=pbs, in_=pb)
                nc.vector.tensor_mul(
                    yT_t[0:D, hp_, :], pav0[0:D, :], pbs[0:D, :]
                )
                nc.vector.tensor_mul(
                    yT_t[D:P, hp_, :], pav1[0:D, :], pbs[D:P, :]
                )

            for q in range(NCHUNK):
                t0 = q * 512
                # ---- QKV for t-chunk q ----
                xT = xT_pool.tile([P, CT, 512], BF16)
                for cc in range(CT):
                    nc.sync.dma_start_transpose(
                        xT[:, cc, :],
                        xb[t0:t0 + 512, cc * P:(cc + 1) * P],
                    )
                # v in [t, j] (+ ones col maintained by init memset)
                for tt in range(4):
                    pv = ps_mm.tile([P, JL], F32, tag="mm")
                    for cc in range(CT):
                        nc.tensor.matmul(
                            pv,
                            lhsT=xT[:, cc, tt * P:(tt + 1) * P],
                            rhs=wT[:, 2, cc, :],
                            start=(cc == 0),
                            stop=(cc == CT - 1),
                        )
                    for h in range(8):
                        nc.vector.tensor_copy(
                            out=v_sb[:, q * 4 + tt, h, 0:D],
                            in_=pv[:, h * D:(h + 1) * D],
                        )
                # q, k in [j, t]
                q_sb = qsb_pool.tile([P, 4, 512], F32R)
                for jt in range(8):
                    g3, j4 = (0, jt) if jt < 4 else (1, jt - 4)
                    pq = ps_mm.tile([P, 512], F32, tag="mm")
                    for cc in range(CT):
                        nc.tensor.matmul(
                            pq,
                            lhsT=wT[:, g3, cc, j4 * P:(j4 + 1) * P],
                            rhs=xT[:, cc, :],
                            start=(cc == 0),
                            stop=(cc == CT - 1),
                        )
                    if jt < 4:
                        nc.vector.tensor_copy(out=q_sb[:, jt, :], in_=pq)
                    else:
                        nc.vector.tensor_copy(
                            out=k_sb[:, jt - 4, t0:t0 + 512], in_=pq
                        )

                # ---- attention for tq-chunk q ----
                yT = yT_pool.tile([P, 4, 512], F32R)
                ntk = 4 * q + 4

                for hp in range(NPAIR):
                    pav0 = ps_av.tile([D + 1, 512], F32, tag="av")
                    pav1 = ps_av.tile([D + 1, 512], F32, tag="av")
                    pav = [pav0, pav1]

                    def emit_scores(j, hp=hp, q_sb=q_sb):
                        # diagonal tiles only need columns o:512
                        diag = j >= 4 * q
                        o = j * P - t0 if diag else 0
                        ps = ps_sc.tile([P, 2, 512], F32, tag="sc")
                        for h2 in range(2):
                            if diag:  # pre-add causal mask into psum
                                nc.tensor.matmul(
                                    ps[:, h2, o:512],
                                    lhsT=identity_r,
                                    rhs=maskr[:, o // P, o:512],
                                    start=True,
                                    stop=False,
                                )
                            nc.tensor.matmul(
                                ps[:, h2, o:512],
                                lhsT=k_sb[
                                    h2 * D:(h2 + 1) * D, hp, j * P:(j + 1) * P
                                ],
                                rhs=q_sb[h2 * D:(h2 + 1) * D, hp, o:512],
                                start=not diag,
                                stop=True,
                            )
                        return ps, o

                    sc_q = [emit_scores(0)]
                    for j in range(ntk):
                        if j + 1 < ntk:
                            sc_q.append(emit_scores(j + 1))
                        ps, o = sc_q[j]
                        pt = pt_pool.tile([P, 2, 512], BF16, tag="pt")
                        nc.scalar.activation(
                            pt[:, :, o:512], ps[:, :, o:512], Exp, scale=0.125
                        )
                        for h2 in range(2):
                            nc.tensor.matmul(
                                pav[h2][:, o:512],
                                lhsT=v_sb[:, j, hp * 2 + h2, :],
                                rhs=pt[:, h2, o:512],
                                start=(j == 0),
                                stop=(j == ntk - 1),
                            )
                    if pending_div is not None:
                        emit_div(pending_div)
                    pending_div = (pav0, pav1, yT, hp)

                # ---- proj for t-chunk q ----
                if pending_div is not None:
                    emit_div(pending_div)
                    pending_div = None
                for tt in range(4):
                    for ct in range(2):
                        po = ps_mm.tile([P, 512], F32, tag="mm")
                        for g in range(4):
                            nc.tensor.matmul(
                                po,
                                lhsT=yT[:, g, tt * P:(tt + 1) * P],
                                rhs=wprojT[:, g, ct * 512:(ct + 1) * 512],
                                start=(g == 0),
                                stop=(g == 3),
                            )
                        ob = ob_pool.tile([P, 512], F32, tag="ob")
                        nc.vector.tensor_copy(ob, po)
                        nc.sync.dma_start(
                            out=out[
                                t0 + tt * P:t0 + (tt + 1) * P,
                                ct * 512:(ct + 1) * 512,
                            ],
                            in_=ob,
                        )

    nc.compile()
    return nc


_NC = None


def _get_nc():
    global _NC
    if _NC is None:
        _NC = build_nc()
    return _NC


def _shard_inputs(x, w_attn, w_proj):
    in_maps = []
    for c in range(8):
        b, s = c // 2, c % 2
        j0 = s * JL
        wqkv_c = np.concatenate(
            [
                w_attn[j0:j0 + JL],
                w_attn[C + j0:C + j0 + JL],
                w_attn[2 * C + j0:2 * C + j0 + JL],
            ],
            axis=0,
        )
        in_maps.append(
            {
                "xb": np.ascontiguousarray(x[b]).astype(ml_dtypes.bfloat16),
                "wqkvb": np.ascontiguousarray(wqkv_c).astype(ml_dtypes.bfloat16),
                "wproj": np.ascontiguousarray(w_proj[:, j0:j0 + JL]).astype(
                    np.float32
                ),
            }
        )
    return in_maps


def run(x, w_attn, w_proj, **run_kwargs):
    """Run on 8 cores; returns (out [B,T,C], BassKernelResults)."""
    nc = _get_nc()
    in_maps = _shard_inputs(np.asarray(x), np.asarray(w_attn), np.asarray(w_proj))
    res = bass_utils.run_bass_kernel_spmd(
        nc, in_maps, core_ids=list(range(8)), **run_kwargs
    )
    out = np.empty((B, T, C), dtype=np.float32)
    for b in range(B):
        out[b] = res.results[2 * b]["out"] + res.results[2 * b + 1]["out"]
    return out, res


def kernel(x, w_attn, w_proj):
    return run(x, w_attn, w_proj)[0]



# revision 2
# speedup vs baseline: 1.7154x; 1.7154x over previous
"""Causal self-attention Bass/Tile kernel for 8 TRN2 NeuronCores.

Sharding: core c handles batch b = c//2 and heads h in [8*(c%2), 8*(c%2)+8).
Each core computes a partial projection output (its 512 channels' worth of the
contraction); the host sums the two partials per batch.

v2 design (software-pipelined, host-pretransposed):
  - Host pre-transposes x -> xT [C,T], wqkv -> wqkvT [C,3*JL], wproj ->
    wprojT [JL,C], all bf16: the kernel issues only plain contiguous DMAs
    (no DMA-transpose, no PE-transpose staging).
  - All of xT + weights resident in SBUF; QKV for chunk q+1 and proj for
    chunk q-1 are emitted as PE "filler" groups interleaved into the
    attention j-loop of chunk q, so the PE never idles while ACT chews exp.
  - Scores: bf16 k/q, K=64 matmul pairs packed into row groups 0-1/2-3
    (base partitions 0/64), fp32 psum [128,2,512] (2 banks), no mask matmul.
  - Causal mask applied post-exp: GpSimd multiplies the 128-wide diagonal
    block of pt by a 0/1 lower-triangle bf16 tile.
  - AV accumulates [65,512] fp32 psum per head (ones column in v gives the
    softmax denominator l in row 64).
  - l broadcast via K=1 f32r select-matmul into a mm-pool psum bank; DVE
    reciprocal + multiply produce yT bf16 for the proj matmuls.
  PSUM budget: scores 2x2 banks + AV 2x1 + mm 2x1 = 8 banks exactly.
"""

import sys
from collections import deque

if "/opt/trn_rl_repo" not in sys.path:
    sys.path.insert(0, "/opt/trn_rl_repo")

import ml_dtypes
import numpy as np

import concourse.bass as bass
import concourse.mybir as mybir
import concourse.tile as tile
from concourse import bacc, bass_utils

F32 = mybir.dt.float32
F32R = mybir.dt.float32r
BF16 = mybir.dt.bfloat16

B, T, C = 4, 2048, 1024
H = 16
D = 64
JL = 512          # local channels per q/k/v slice (8 heads * 64)
P = 128
NCHUNK = T // 512
CT = C // P       # 8 c-tiles


def build_nc():
    nc = bacc.Bacc("TRN2", target_bir_lowering=False, debug=False)
    xT_d = nc.dram_tensor("xT", [C, T], BF16, kind="ExternalInput").ap()
    wqkvT_d = nc.dram_tensor("wqkvT", [C, 3 * JL], BF16, kind="ExternalInput").ap()
    wprojT_d = nc.dram_tensor("wprojT", [JL, C], BF16, kind="ExternalInput").ap()
    out_d = nc.dram_tensor("out", [T, C], F32, kind="ExternalOutput").ap()

    Exp = mybir.ActivationFunctionType.Exp

    with tile.TileContext(nc) as tc:
        with (
            tc.tile_pool(name="singles", bufs=1) as singles,
            tc.tile_pool(name="qsb", bufs=2) as qsb,
            tc.tile_pool(name="ptp", bufs=4) as ptp,
            tc.tile_pool(name="ytp", bufs=2) as ytp,
            tc.tile_pool(name="obp", bufs=3) as obp,
            tc.tile_pool(name="smp", bufs=4) as smp,
            tc.tile_pool(name="ps_sc", bufs=2, space="PSUM") as ps_sc,
            tc.tile_pool(name="ps_av", bufs=2, space="PSUM") as ps_av,
            tc.tile_pool(name="ps_mm", bufs=2, space="PSUM") as ps_mm,
        ):
            # ---- persistent SBUF tensors ----
            xT_sb = singles.tile([P, CT, T], BF16)        # 32KB/part
            wq_sb = singles.tile([P, CT, 3 * JL], BF16)   # 24KB/part
            wp_sb = singles.tile([P, 4, C], BF16)         # 8KB/part
            k_sb = singles.tile([P, 4, T], BF16)          # 16KB/part
            v_sb = singles.tile([P, T // P, 8, D + 1], BF16)
            nc.vector.memset(v_sb[:, :, :, D], 1.0)

            # head-pair selector for the l broadcast: out rows 0:64 <- l0,
            # rows 64:128 <- l1  (out = sel.T @ [l0;l1])
            selaf = singles.tile([1, P], F32)
            nc.vector.memset(selaf, 0.0)
            nc.vector.memset(selaf[0:1, 0:D], 1.0)
            selbf = singles.tile([1, P], F32)
            nc.vector.memset(selbf, 0.0)
            nc.vector.memset(selbf[0:1, D:2 * D], 1.0)
            sel_a = singles.tile([1, P], F32R)
            nc.vector.tensor_copy(sel_a, selaf)
            sel_b = singles.tile([1, P], F32R)
            nc.vector.tensor_copy(sel_b, selbf)

            # 0/1 lower-triangle mask (keep f >= p), both h2 slots
            tri = singles.tile([P, 2, P], BF16)
            nc.gpsimd.memset(tri, 1.0)
            nc.gpsimd.affine_select(
                out=tri,
                in_=tri,
                pattern=[[0, 2], [1, P]],
                compare_op=mybir.AluOpType.is_ge,
                fill=0.0,
                base=0,
                channel_multiplier=-1,
            )

            # ---- input DMAs (plain, contiguous) ----
            for cc in range(CT):
                nc.sync.dma_start(
                    out=wq_sb[:, cc, :], in_=wqkvT_d[cc * P:(cc + 1) * P, :]
                )
                nc.sync.dma_start(
                    out=xT_sb[:, cc, :], in_=xT_d[cc * P:(cc + 1) * P, :]
                )
            for g in range(4):
                nc.sync.dma_start(
                    out=wp_sb[:, g, :], in_=wprojT_d[g * P:(g + 1) * P, :]
                )

            # ---- emission helpers ----
            q_tiles = {}
            yT_tiles = {}

            def emit_qk_group(q, hp, qt):
                t0 = q * 512
                for kind in range(2):            # 0 = q, 1 = k
                    col0 = kind * JL + hp * P
                    pq = ps_mm.tile([P, 512], F32, tag="mm", name="pq")
                    for cc in range(CT):
                        nc.tensor.matmul(
                            pq,
                            lhsT=wq_sb[:, cc, col0:col0 + P],
                            rhs=xT_sb[:, cc, t0:t0 + 512],
                            start=(cc == 0),
                            stop=(cc == CT - 1),
                        )
                    if kind == 0:
                        nc.vector.tensor_copy(out=qt[:, hp, :], in_=pq)
                    else:
                        nc.vector.tensor_copy(
                            out=k_sb[:, hp, t0:t0 + 512], in_=pq
                        )

            def emit_v_group(q, tt):
                t0 = q * 512
                pv = ps_mm.tile([P, 8, D], F32, tag="mm", name="pv")
                for cc in range(CT):
                    nc.tensor.matmul(
                        pv,
                        lhsT=xT_sb[:, cc, t0 + tt * P:t0 + (tt + 1) * P],
                        rhs=wq_sb[:, cc, 2 * JL:3 * JL],
                        start=(cc == 0),
                        stop=(cc == CT - 1),
                    )
                nc.vector.tensor_copy(
                    out=v_sb[:, q * 4 + tt, :, 0:D], in_=pv
                )

            def make_qkv_fillers(q):
                qt = qsb.tile([P, 4, 512], BF16, tag="q", name="qt")
                q_tiles[q] = qt
                fns = [
                    (lambda hp=hp: emit_qk_group(q, hp, qt)) for hp in range(4)
                ]
                fns += [(lambda tt=tt: emit_v_group(q, tt)) for tt in range(4)]
                return fns

            def emit_proj_group(q, tt, ct):
                t0 = q * 512
                yT_t = yT_tiles[q]
                po = ps_mm.tile([P, 512], F32, tag="mm", name="po")
                for g in range(4):
                    nc.tensor.matmul(
                        po,
                        lhsT=yT_t[:, g, tt * P:(tt + 1) * P],
                        rhs=wp_sb[:, g, ct * 512:(ct + 1) * 512],
                        start=(g == 0),
                        stop=(g == 3),
                    )
                obt = obp.tile([P, 512], F32, tag="ob", name="obt")
                nc.vector.tensor_copy(obt, po)
                nc.sync.dma_start(
                    out=out_d[
                        t0 + tt * P:t0 + (tt + 1) * P,
                        ct * 512:(ct + 1) * 512,
                    ],
                    in_=obt,
                )

            def make_proj_fillers(q):
                return [
                    (lambda tt=tt, ct=ct: emit_proj_group(q, tt, ct))
                    for tt in range(4) for ct in range(2)
                ]

            filler = deque()
            fstate = {"acc": 0.0, "rate": 0.0}

            def maybe_filler():
                fstate["acc"] += fstate["rate"]
                while fstate["acc"] >= 1.0 and filler:
                    filler.popleft()()
                    fstate["acc"] -= 1.0

            def flush_fillers():
                while filler:
                    filler.popleft()()
                fstate["acc"] = 0.0

            def emit_attention(q):
                ntk = 4 * (q + 1)
                t0 = q * 512
                qt = q_tiles[q]
                yT_t = ytp.tile([P, 4, 512], BF16, tag="yT", name="yT_t")
                yT_tiles[q] = yT_t
                # pacing: distribute current fillers over this stage's slots
                slots = 4 * (ntk + 3)
                fstate["rate"] = len(filler) / slots if slots else 0.0
                fstate["acc"] = 0.0

                for hp in range(4):
                    pav = [
                        ps_av.tile([D + 1, 512], F32, tag="av", name="pav")
                        for _ in range(2)
                    ]
                    cur = {}

                    def emit_sc_exp(j, hp=hp, qt=qt, cur=cur):
                        diag = j >= 4 * q
                        o = j * P - t0 if diag else 0
                        ps = ps_sc.tile([P, 2, 512], F32, tag="sc", name="ps")
                        for h2 in range(2):
                            nc.tensor.matmul(
                                ps[:, h2, o:512],
                                lhsT=k_sb[
                                    h2 * D:(h2 + 1) * D, hp, j * P:(j + 1) * P
                                ],
                                rhs=qt[h2 * D:(h2 + 1) * D, hp, o:512],
                                start=True,
                                stop=True,
                            )
                        pt = ptp.tile([P, 2, 512], BF16, tag="pt", name="pt")
                        nc.scalar.activation(
                            pt[:, :, o:512], ps[:, :, o:512], Exp, scale=0.125
                        )
                        if diag:
                            nc.gpsimd.tensor_mul(
                                pt[:, :, o:o + P], pt[:, :, o:o + P], tri
                            )
                        cur[j] = (pt, o)

                    emit_sc_exp(0)
                    for j in range(ntk):
                        if j + 1 < ntk:
                            emit_sc_exp(j + 1)
                        maybe_filler()
                        pt, o = cur.pop(j)
                        for h2 in range(2):
                            nc.tensor.matmul(
                                pav[h2][:, o:512],
                                lhsT=v_sb[:, j, hp * 2 + h2, :],
                                rhs=pt[:, h2, o:512],
                                start=(j == 0),
                                stop=(j == ntk - 1),
                            )

                    # softmax divide: l rows -> broadcast -> reciprocal -> mul
                    l2a = smp.tile([1, 512], F32R, tag="l2", name="l2a")
                    nc.vector.tensor_copy(l2a, pav[0][D:D + 1, :])
                    l2b = smp.tile([1, 512], F32R, tag="l2", name="l2b")
                    nc.vector.tensor_copy(l2b, pav[1][D:D + 1, :])
                    maybe_filler()
                    pb = ps_mm.tile([P, 512], F32, tag="mm", name="pb")
                    nc.tensor.matmul(pb, lhsT=sel_a, rhs=l2a,
                                     start=True, stop=False)
                    nc.tensor.matmul(pb, lhsT=sel_b, rhs=l2b,
                                     start=False, stop=True)
                    maybe_filler()
                    pbs = smp.tile([P, 512], F32, tag="pbs", name="pbs")
                    nc.vector.reciprocal_approx_fast(out=pbs, in_=pb)
                    nc.vector.tensor_mul(
                        yT_t[0:D, hp, :], pav[0][0:D, :], pbs[0:D, :]
                    )
                    nc.vector.tensor_mul(
                        yT_t[D:P, hp, :], pav[1][0:D, :], pbs[D:P, :]
                    )
                    maybe_filler()

            # ---- main schedule ----
            f0 = make_qkv_fillers(0)
            f0[0]()                       # qk group hp0 (critical path)
            for tt in range(4):
                f0[4 + tt]()              # v groups chunk 0
            filler.extend(f0[1:4])        # remaining qk groups chunk 0
            filler.extend(make_qkv_fillers(1))
            emit_attention(0)
            flush_fillers()

            for q in range(1, NCHUNK):
                filler.extend(make_proj_fillers(q - 1))
                if q + 1 < NCHUNK:
                    filler.extend(make_qkv_fillers(q + 1))
                emit_attention(q)
                flush_fillers()

            for fn in make_proj_fillers(NCHUNK - 1):
                fn()

    nc.compile()
    return nc


_NC = None


def _get_nc():
    global _NC
    if _NC is None:
        _NC = build_nc()
    return _NC


def _shard_inputs(x, w_attn, w_proj):
    bf16 = ml_dtypes.bfloat16
    xT_b = [
        np.ascontiguousarray(x[b].T).astype(bf16) for b in range(B)
    ]
    wq_s, wp_s = [], []
    for s in range(2):
        j0 = s * JL
        blocks = [w_attn[g * C + j0:g * C + j0 + JL, :] for g in range(3)]
        wq = np.concatenate(blocks, axis=0)            # [3*JL, C]
        wq_s.append(np.ascontiguousarray(wq.T).astype(bf16))   # [C, 3*JL]
        wp_s.append(
            np.ascontiguousarray(w_proj[:, j0:j0 + JL].T).astype(bf16)
        )                                              # [JL, C]
    return [
        {
            "xT": xT_b[c // 2],
            "wqkvT": wq_s[c % 2],
            "wprojT": wp_s[c % 2],
        }
        for c in range(8)
    ]


def run(x, w_attn, w_proj, **run_kwargs):
    """Run on 8 cores; returns (out [B,T,C], BassKernelResults)."""
    nc = _get_nc()
    in_maps = _shard_inputs(np.asarray(x), np.asarray(w_attn), np.asarray(w_proj))
    res = bass_utils.run_bass_kernel_spmd(
        nc, in_maps, core_ids=list(range(8)), **run_kwargs
    )
    out = np.empty((B, T, C), dtype=np.float32)
    for b in range(B):
        out[b] = res.results[2 * b]["out"] + res.results[2 * b + 1]["out"]
    return out, res


def kernel(x, w_attn, w_proj):
    return run(x, w_attn, w_proj)[0]
